# revision 1
# baseline (speedup 1.0000x reference)
"""Batched Kalman filter update on 8 trn2 NeuronCores.

Strategy (pure data parallel over the track dim B=262144, 32768/core):
- Natural layout in SBUF: partition = batch-high, per-partition columns hold
  each element's [x(8) | z(4) | P(64) | 1] = 77 floats contiguously.
- TensorE bridge per 128-element block: transpose [128,77] -> [77,128]
  (entries-on-partitions), then one fp32 matmul with a host-baked weight
  matrix W1 [77,46] computes U=P.H^T (32), S=H.P.H^T+R upper (10), y=z-Hx (4)
  straight back in natural layout [128,46].
- VectorE solves the 4x4 SPD system via LDL^T per element with strided /
  broadcast access patterns (all ops [128, nq, w], full 128-lane utilization):
    S = L D L^T;  W = U L^-T;  v = L^-1 y
    x_new = x + sum_j w_j * v_j / d_j
    P_new = P - sum_j (w_j/sqrt(d_j)) (w_j/sqrt(d_j))^T   (upper + mirror)
  Algebraically identical to K = U S^-1 (Joseph form equals this exactly).
"""

import numpy as np

import concourse.bacc as bacc
import concourse.tile as tile
from concourse import mybir

NCORES = 8
B = 262144
BC = B // NCORES          # 32768 per core
P128 = 128
N = BC // P128            # 256 elements per partition
Q = 2                     # sub-chunks for DMA/compute overlap
NQ = N // Q               # 128 columns per sub-chunk

F32 = mybir.dt.float32
MUL = mybir.AluOpType.mult
SUB = mybir.AluOpType.subtract
ADD = mybir.AluOpType.add

# upper-triangle index order for S (4x4): (m,l) m<=l
SUP = [(0, 0), (0, 1), (0, 2), (0, 3), (1, 1), (1, 2), (1, 3), (2, 2), (2, 3), (3, 3)]


def _build_w1(H: np.ndarray, R: np.ndarray) -> np.ndarray:
    """W1 [77, 46]: rows = [x(0:8) | z(8:12) | P(12:76) | ones(76)],
    cols = [U(i*4+m) 0:32 | S upper 32:42 | y 42:46]."""
    W1 = np.zeros((77, 46), dtype=np.float32)
    # U[i,m] = sum_k P[i,k] H[m,k]
    for i in range(8):
        for m in range(4):
            for k in range(8):
                W1[12 + i * 8 + k, i * 4 + m] = H[m, k]
    # S[m,l] = sum_{i,k} H[m,i] H[l,k] P[i,k] + R[m,l]
    for idx, (m, l) in enumerate(SUP):
        for i in range(8):
            for k in range(8):
                W1[12 + i * 8 + k, 32 + idx] += H[m, i] * H[l, k]
        W1[76, 32 + idx] = R[m, l]
    # y_m = z_m - sum_k H[m,k] x_k
    for m in range(4):
        W1[8 + m, 42 + m] = 1.0
        for k in range(8):
            W1[k, 42 + m] = -H[m, k]
    return W1


def _build_program():
    nc = bacc.Bacc("TRN2", target_bir_lowering=False, debug=False,
                   num_devices=NCORES)
    xd = nc.dram_tensor("xd", [BC, 8], F32, kind="ExternalInput")
    zd = nc.dram_tensor("zd", [BC, 4], F32, kind="ExternalInput")
    Pd = nc.dram_tensor("Pd", [BC, 64], F32, kind="ExternalInput")
    w1d = nc.dram_tensor("w1d", [77, 46], F32, kind="ExternalInput")
    idd = nc.dram_tensor("idd", [128, 128], F32, kind="ExternalInput")
    outd = nc.dram_tensor("outd", [BC, 72], F32, kind="ExternalOutput")

    xv = xd.ap().rearrange("(p f) c -> p f c", p=P128)    # [128, N, 8]
    zv = zd.ap().rearrange("(p f) c -> p f c", p=P128)
    Pv = Pd.ap().rearrange("(p f) c -> p f c", p=P128)
    ov = outd.ap().rearrange("(p f) c -> p f c", p=P128)  # [128, N, 72]

    with tile.TileContext(nc) as tc:
        with (
            tc.tile_pool(name="consts", bufs=1) as consts,
            tc.tile_pool(name="xpz", bufs=2) as xpz_pool,
            tc.tile_pool(name="ut", bufs=2) as ut_pool,
            tc.tile_pool(name="sc", bufs=2) as sc_pool,
            tc.tile_pool(name="xvrt", bufs=3) as xv_pool,
            tc.tile_pool(name="tps", bufs=3, space="PSUM") as tp_ps,
            tc.tile_pool(name="usps", bufs=3, space="PSUM") as us_ps,
        ):
            w1s = consts.tile([77, 46], F32)
            nc.sync.dma_start(out=w1s, in_=w1d.ap())
            ids = consts.tile([128, 128], F32)
            nc.sync.dma_start(out=ids, in_=idd.ap())

            for q in range(Q):
                f0 = q * NQ
                FS = slice(f0, f0 + NQ)

                XPZ = xpz_pool.tile([P128, NQ, 77], F32, tag="xpz")
                UT = ut_pool.tile([P128, NQ, 46], F32, tag="ut")
                SC = sc_pool.tile([P128, NQ, 26], F32, tag="sc")

                nc.sync.dma_start(out=XPZ[:, :, 0:8], in_=xv[:, FS, :])
                nc.sync.dma_start(out=XPZ[:, :, 8:12], in_=zv[:, FS, :])
                nc.sync.dma_start(out=XPZ[:, :, 12:76], in_=Pv[:, FS, :])
                nc.vector.memset(XPZ[:, :, 76:77], 1.0)

                # ---- TensorE bridge: transpose + linear pass, 4 blocks/copy ----
                for f in range(0, NQ, 4):
                    tp = tp_ps.tile([77, 512], F32, tag="tp")
                    for g in range(4):
                        nc.tensor.transpose(tp[:, g * 128:(g + 1) * 128],
                                            XPZ[:, f + g, :], ids)
                    xvert = xv_pool.tile([77, 512], F32, tag="xvert")
                    nc.scalar.copy(xvert, tp)
                    us = us_ps.tile([128, 184], F32, tag="us")
                    for g in range(4):
                        nc.tensor.matmul(us[:, g * 46:(g + 1) * 46],
                                         xvert[:, g * 128:(g + 1) * 128], w1s)
                    nc.scalar.copy(UT[:, f:f + 4, :],
                                   us.rearrange("p (f c) -> p f c", f=4))

                # ---- helpers -------------------------------------------
                def U(c0, w=1):
                    return UT[:, :, c0:c0 + w]

                def S(c0, w=1):
                    return SC[:, :, c0:c0 + w]

                def bc(ap, w):
                    return ap.broadcast_to([P128, NQ, w])

                tmp = SC[:, :, 18:26]       # 8-wide scratch

                def T(out, a, b, op):
                    nc.vector.tensor_tensor(out=out, in0=a, in1=b, op=op)

                # ---- LDL of S (in place in UT cols 32..41) -------------
                # cols: s00=32 s01=33 s02=34 s03=35 s11=36 s12=37 s13=38
                #       s22=39 s23=40 s33=41 ; y/v = 42..45
                nc.vector.reciprocal(S(6), U(32))                # rec0
                T(S(0, 3), U(33, 3), bc(S(6), 3), MUL)           # l10,l20,l30
                T(tmp[:, :, 0:3], bc(S(0), 3), U(33, 3), MUL)
                T(U(36, 3), U(36, 3), tmp[:, :, 0:3], SUB)       # s11,s12,s13
                T(tmp[:, :, 0:2], bc(S(1), 2), U(34, 2), MUL)
                T(U(39, 2), U(39, 2), tmp[:, :, 0:2], SUB)       # s22,s23
                T(tmp[:, :, 0:1], S(2), U(35), MUL)
                T(U(41), U(41), tmp[:, :, 0:1], SUB)             # s33
                nc.vector.reciprocal(S(7), U(36))                # rec1
                T(S(3, 2), U(37, 2), bc(S(7), 2), MUL)           # l21,l31
                T(tmp[:, :, 0:2], bc(S(3), 2), U(37, 2), MUL)
                T(U(39, 2), U(39, 2), tmp[:, :, 0:2], SUB)
                T(tmp[:, :, 0:1], S(4), U(38), MUL)
                T(U(41), U(41), tmp[:, :, 0:1], SUB)
                nc.vector.reciprocal(S(8), U(39))                # rec2
                T(S(5), U(40), S(8), MUL)                        # l32
                T(tmp[:, :, 0:1], S(5), U(40), MUL)
                T(U(41), U(41), tmp[:, :, 0:1], SUB)
                nc.vector.reciprocal(S(9), U(41))                # rec3
                nc.scalar.activation(S(10, 4), S(6, 4),
                                     mybir.ActivationFunctionType.Sqrt)

                # ---- v = L^-1 y (in place in UT 42..45), atil ----------
                T(tmp[:, :, 0:3], S(0, 3), bc(U(42), 3), MUL)
                T(U(43, 3), U(43, 3), tmp[:, :, 0:3], SUB)
                T(tmp[:, :, 0:2], S(3, 2), bc(U(43), 2), MUL)
                T(U(44, 2), U(44, 2), tmp[:, :, 0:2], SUB)
                T(tmp[:, :, 0:1], S(5), U(44), MUL)
                T(U(45), U(45), tmp[:, :, 0:1], SUB)
                T(S(14, 4), U(42, 4), S(10, 4), MUL)             # atil = v*sqrtrec

                # ---- W solve in place over U cols ----------------------
                Uv = UT[:, :, 0:32].rearrange("p f (i m) -> p f i m", m=4)

                def um(m):
                    return Uv[:, :, :, m]                        # [128,NQ,8] stride 4

                for (m, j, lc) in ((1, 0, 0), (2, 0, 1), (2, 1, 3),
                                   (3, 0, 2), (3, 1, 4), (3, 2, 5)):
                    # u_m -= l(m,j) * w_j
                    T(tmp, um(j), bc(S(lc), 8), MUL)
                    T(um(m), um(m), tmp, SUB)
                for j in range(4):                                # scale: wtil
                    T(um(j), um(j), bc(S(10 + j), 8), MUL)

                # ---- x update ------------------------------------------
                X = XPZ[:, :, 0:8]
                for j in range(4):
                    T(tmp, um(j), bc(S(14 + j), 8), MUL)
                    T(X, X, tmp, ADD)

                # ---- P update (upper), then mirror ---------------------
                for j in range(4):
                    for i in range(8):
                        w = 8 - i
                        lhs = bc(UT[:, :, i * 4 + j:i * 4 + j + 1], w)
                        rhs = Uv[:, :, i:8, j]
                        T(tmp[:, :, 0:w], lhs, rhs, MUL)
                        prun = XPZ[:, :, 12 + i * 8 + i: 12 + i * 8 + 8]
                        T(prun, prun, tmp[:, :, 0:w], SUB)

                P2 = XPZ[:, :, 12:76].rearrange("p f (i k) -> p f i k", i=8)
                for i in range(1, 8):
                    nc.scalar.copy(P2[:, :, i, 0:i], P2[:, :, 0:i, i])

                # ---- DMA out -------------------------------------------
                nc.sync.dma_start(out=ov[:, FS, 0:8], in_=XPZ[:, :, 0:8])
                nc.sync.dma_start(out=ov[:, FS, 8:72], in_=XPZ[:, :, 12:76])

    nc.compile()
    return nc


_prog_cache = {}


def kernel(x: np.ndarray, z: np.ndarray, P: np.ndarray,
           H: np.ndarray, R: np.ndarray) -> np.ndarray:
    from concourse.bass_utils import run_bass_kernel_spmd

    x = np.ascontiguousarray(x, dtype=np.float32).reshape(B, 8)
    z = np.ascontiguousarray(z, dtype=np.float32).reshape(B, 4)
    P = np.ascontiguousarray(P, dtype=np.float32).reshape(B, 64)
    W1 = _build_w1(np.asarray(H, np.float32), np.asarray(R, np.float32))
    ident = np.eye(128, dtype=np.float32)

    if "nc" not in _prog_cache:
        _prog_cache["nc"] = _build_program()
    nc = _prog_cache["nc"]

    in_maps = []
    for c in range(NCORES):
        s = slice(c * BC, (c + 1) * BC)
        in_maps.append({"xd": x[s], "zd": z[s], "Pd": P[s],
                        "w1d": W1, "idd": ident})
    res = run_bass_kernel_spmd(nc, in_maps, core_ids=list(range(NCORES)))
    out = np.concatenate([r["outd"].reshape(BC, 9, 8) for r in res.results],
                         axis=0)
    return out



# revision 8
# speedup vs baseline: 2.9122x; 2.9122x over previous
"""Batched Kalman filter update on 8 trn2 NeuronCores (axon-tunneled).

The end-to-end wall clock is dominated by the ~50 MB/s axon tunnel, so the
design minimizes wire bytes and per-call overhead:

Host side (fp32, exact):
- y = z - H x (one BLAS gemm) so x/z never go to the device.
- P is SPD: ship only the packed upper triangle (36 of 64 cols), fp16.
- Unpack: x_new = x + dx (device fp16 delta), P_new mirrored from the
  packed upper triangle the device returns. Total wire ~42 MB/call vs
  ~220 MB for the naive fp32 full-tensor + zero-donation path.

Device side (per core, data parallel over the track dim):
- DMA packed fp16 [y(4) | Pu(36)], upcast once to fp32 in SBUF.
- TensorE bridge per 128-block: transpose [128,37] -> [37,128]
  (entries-on-partitions) with an identity matmul, then one fp32 matmul
  with host-baked W2 [37,42] = U = P H^T (32 cols) and the upper
  triangle of S = H P H^T + R (10 cols), back in natural layout.
- VectorE solves the 4x4 SPD system per element via LDL^T (all ops
  [128, nq, w], full 128-lane utilization):
    S = L D L^T;  W = U L^-T;  v = L^-1 y
    dx    = sum_j w_j v_j / d_j
    P_new = P - sum_j (w_j/sqrt(d_j)) (w_j/sqrt(d_j))^T  (upper only)
  Algebraically identical to K = U S^-1 / Joseph form.
- Write [dx(8) | P_new upper(36)] fp16, one DMA out.

Runner: a cached jax.jit(shard_map) over the bass_exec primitive (the
same machinery bass_utils.run_bass_kernel_spmd uses under axon), built
once per process; constants (identity, W2) live on device; no zero
output buffers are shipped (the kernel writes every output element);
output shards are fetched with parallel threads (the tunnel single
stream is ~28 MB/s but ~50 MB/s with 8 streams).
"""

import numpy as np
from concurrent.futures import ThreadPoolExecutor

import concourse.bacc as bacc
import concourse.tile as tile
from concourse import mybir

NCORES = 8
B = 262144
CHUNKS = 1                  # host-level pipeline chunks per call
BC = B // NCORES // CHUNKS  # rows per core per chunk
P128 = 128
N = BC // P128              # elements per partition
Q = 2                       # sub-chunks for DMA/compute overlap
NQ = N // Q

F32 = mybir.dt.float32
F16 = mybir.dt.float16
MUL = mybir.AluOpType.mult
SUB = mybir.AluOpType.subtract
ADD = mybir.AluOpType.add

# upper-triangle index order for S (4x4): (m,l) m<=l
SUP = [(0, 0), (0, 1), (0, 2), (0, 3), (1, 1), (1, 2), (1, 3), (2, 2), (2, 3), (3, 3)]
# packed upper triangle of P (8x8), row-major: (a,b) a<=b
PUP = [(a, b) for a in range(8) for b in range(a, 8)]
IU = np.array([a * 8 + b for a, b in PUP], dtype=np.intp)        # 36 full-cols
OFF = [0, 8, 15, 21, 26, 30, 33, 35]                             # row starts
# full 64 P cols -> packed col
FULLIDX = np.array([OFF[min(i, k)] + abs(k - i)
                    for i in range(8) for k in range(8)], dtype=np.intp)


def _build_w1(H: np.ndarray, R: np.ndarray) -> np.ndarray:
    """W1 [77, 46]: rows = [x(0:8) | z(8:12) | P(12:76) | ones(76)],
    cols = [U(i*4+m) 0:32 | S upper 32:42 | y 42:46]."""
    W1 = np.zeros((77, 46), dtype=np.float32)
    for i in range(8):
        for m in range(4):
            for k in range(8):
                W1[12 + i * 8 + k, i * 4 + m] = H[m, k]
    for idx, (m, l) in enumerate(SUP):
        for i in range(8):
            for k in range(8):
                W1[12 + i * 8 + k, 32 + idx] += H[m, i] * H[l, k]
        W1[76, 32 + idx] = R[m, l]
    for m in range(4):
        W1[8 + m, 42 + m] = 1.0
        for k in range(8):
            W1[k, 42 + m] = -H[m, k]
    return W1


def _build_w2(H: np.ndarray, R: np.ndarray) -> np.ndarray:
    """W2 [37, 42]: rows = [packed upper P (36) | ones], cols = [U 0:32 |
    S upper 32:42]. Folded from W1 by symmetry P[a,b] == P[b,a]."""
    W1 = _build_w1(H, R)
    W2 = np.zeros((37, 42), dtype=np.float32)
    for m, (a, b) in enumerate(PUP):
        row = W1[12 + a * 8 + b, 0:42].copy()
        if a != b:
            row += W1[12 + b * 8 + a, 0:42]
        W2[m] = row
    W2[36] = W1[76, 0:42]
    return W2


def _build_program(bc: int):
    n = bc // P128
    nq = n // Q
    assert nq % 4 == 0

    nc = bacc.Bacc("TRN2", target_bir_lowering=False, debug=False,
                   num_devices=NCORES)
    yd = nc.dram_tensor("yd", [bc, 4], F16, kind="ExternalInput")
    pud = nc.dram_tensor("pud", [bc, 36], F16, kind="ExternalInput")
    w2d = nc.dram_tensor("w2d", [37, 42], F32, kind="ExternalInput")
    idd = nc.dram_tensor("idd", [128, 128], F32, kind="ExternalInput")
    outd = nc.dram_tensor("outd", [bc, 44], F16, kind="ExternalOutput")

    yv = yd.ap().rearrange("(p f) c -> p f c", p=P128)    # [128, n, 4]
    pv = pud.ap().rearrange("(p f) c -> p f c", p=P128)   # [128, n, 36]
    ov = outd.ap().rearrange("(p f) c -> p f c", p=P128)  # [128, n, 44]

    with tile.TileContext(nc) as tc:
        with (
            tc.tile_pool(name="consts", bufs=1) as consts,
            tc.tile_pool(name="ypu", bufs=2) as ypu_pool,
            tc.tile_pool(name="xpz", bufs=2) as xpz_pool,
            tc.tile_pool(name="ut", bufs=2) as ut_pool,
            tc.tile_pool(name="sc", bufs=2) as sc_pool,
            tc.tile_pool(name="dxo", bufs=2) as dxo_pool,
            tc.tile_pool(name="xvrt", bufs=3) as xv_pool,
            tc.tile_pool(name="tps", bufs=3, space="PSUM") as tp_ps,
            tc.tile_pool(name="usps", bufs=3, space="PSUM") as us_ps,
        ):
            w2s = consts.tile([37, 42], F32)
            nc.sync.dma_start(out=w2s, in_=w2d.ap())
            ids = consts.tile([128, 128], F32)
            nc.sync.dma_start(out=ids, in_=idd.ap())

            for q in range(Q):
                f0 = q * nq
                FS = slice(f0, f0 + nq)

                YPU = ypu_pool.tile([P128, nq, 40], F16, tag="ypu")
                XPZ = xpz_pool.tile([P128, nq, 41], F32, tag="xpz")
                UT = ut_pool.tile([P128, nq, 46], F32, tag="ut")
                SC = sc_pool.tile([P128, nq, 26], F32, tag="sc")
                DX = dxo_pool.tile([P128, nq, 8], F32, tag="dx")
                OUTB = dxo_pool.tile([P128, nq, 44], F16, tag="outb")

                nc.sync.dma_start(out=YPU[:, :, 0:4], in_=yv[:, FS, :])
                nc.sync.dma_start(out=YPU[:, :, 4:40], in_=pv[:, FS, :])
                # upcast fp16 -> fp32 once; cols: [y(0:4) | Pu(4:40) | 1]
                nc.scalar.copy(XPZ[:, :, 0:40], YPU)
                nc.vector.memset(XPZ[:, :, 40:41], 1.0)

                # ---- TensorE bridge: transpose + linear pass, 4 blocks ----
                for f in range(0, nq, 4):
                    tp = tp_ps.tile([37, 512], F32, tag="tp")
                    for g in range(4):
                        nc.tensor.transpose(tp[:, g * 128:(g + 1) * 128],
                                            XPZ[:, f + g, 4:41], ids)
                    xvert = xv_pool.tile([37, 512], F32, tag="xvert")
                    nc.scalar.copy(xvert, tp)
                    us = us_ps.tile([128, 168], F32, tag="us")
                    for g in range(4):
                        nc.tensor.matmul(us[:, g * 42:(g + 1) * 42],
                                         xvert[:, g * 128:(g + 1) * 128], w2s)
                    nc.scalar.copy(UT[:, f:f + 4, 0:42],
                                   us.rearrange("p (f c) -> p f c", f=4))
                # y into the solve slot (UT cols 42:46)
                nc.scalar.copy(UT[:, :, 42:46], XPZ[:, :, 0:4])

                # ---- helpers -------------------------------------------
                def U(c0, w=1):
                    return UT[:, :, c0:c0 + w]

                def S(c0, w=1):
                    return SC[:, :, c0:c0 + w]

                def bc_(ap, w):
                    return ap.broadcast_to([P128, nq, w])

                tmp = SC[:, :, 18:26]       # 8-wide scratch

                def T(out, a, b, op):
                    nc.vector.tensor_tensor(out=out, in0=a, in1=b, op=op)

                # ---- LDL of S (in place in UT cols 32..41) -------------
                # cols: s00=32 s01=33 s02=34 s03=35 s11=36 s12=37 s13=38
                #       s22=39 s23=40 s33=41 ; y/v = 42..45
                nc.vector.reciprocal(S(6), U(32))                # rec0
                T(S(0, 3), U(33, 3), bc_(S(6), 3), MUL)          # l10,l20,l30
                T(tmp[:, :, 0:3], bc_(S(0), 3), U(33, 3), MUL)
                T(U(36, 3), U(36, 3), tmp[:, :, 0:3], SUB)       # s11,s12,s13
                T(tmp[:, :, 0:2], bc_(S(1), 2), U(34, 2), MUL)
                T(U(39, 2), U(39, 2), tmp[:, :, 0:2], SUB)       # s22,s23
                T(tmp[:, :, 0:1], S(2), U(35), MUL)
                T(U(41), U(41), tmp[:, :, 0:1], SUB)             # s33
                nc.vector.reciprocal(S(7), U(36))                # rec1
                T(S(3, 2), U(37, 2), bc_(S(7), 2), MUL)          # l21,l31
                T(tmp[:, :, 0:2], bc_(S(3), 2), U(37, 2), MUL)
                T(U(39, 2), U(39, 2), tmp[:, :, 0:2], SUB)
                T(tmp[:, :, 0:1], S(4), U(38), MUL)
                T(U(41), U(41), tmp[:, :, 0:1], SUB)
                nc.vector.reciprocal(S(8), U(39))                # rec2
                T(S(5), U(40), S(8), MUL)                        # l32
                T(tmp[:, :, 0:1], S(5), U(40), MUL)
                T(U(41), U(41), tmp[:, :, 0:1], SUB)
                nc.vector.reciprocal(S(9), U(41))                # rec3
                nc.scalar.activation(S(10, 4), S(6, 4),
                                     mybir.ActivationFunctionType.Sqrt)

                # ---- v = L^-1 y (in place in UT 42..45), atil ----------
                T(tmp[:, :, 0:3], S(0, 3), bc_(U(42), 3), MUL)
                T(U(43, 3), U(43, 3), tmp[:, :, 0:3], SUB)
                T(tmp[:, :, 0:2], S(3, 2), bc_(U(43), 2), MUL)
                T(U(44, 2), U(44, 2), tmp[:, :, 0:2], SUB)
                T(tmp[:, :, 0:1], S(5), U(44), MUL)
                T(U(45), U(45), tmp[:, :, 0:1], SUB)
                T(S(14, 4), U(42, 4), S(10, 4), MUL)             # atil

                # ---- W solve in place over U cols ----------------------
                Uv = UT[:, :, 0:32].rearrange("p f (i m) -> p f i m", m=4)

                def um(m):
                    return Uv[:, :, :, m]                        # [128,nq,8]

                for (m, j, lc) in ((1, 0, 0), (2, 0, 1), (2, 1, 3),
                                   (3, 0, 2), (3, 1, 4), (3, 2, 5)):
                    T(tmp, um(j), bc_(S(lc), 8), MUL)
                    T(um(m), um(m), tmp, SUB)
                for j in range(4):                                # scale: wtil
                    T(um(j), um(j), bc_(S(10 + j), 8), MUL)

                # ---- dx = sum_j wtil_j * atil_j ------------------------
                T(DX, um(0), bc_(S(14), 8), MUL)
                for j in range(1, 4):
                    T(tmp, um(j), bc_(S(14 + j), 8), MUL)
                    T(DX, DX, tmp, ADD)

                # ---- P update (packed upper triangle) ------------------
                for j in range(4):
                    for i in range(8):
                        w = 8 - i
                        lhs = bc_(UT[:, :, i * 4 + j:i * 4 + j + 1], w)
                        rhs = Uv[:, :, i:8, j]
                        T(tmp[:, :, 0:w], lhs, rhs, MUL)
                        prun = XPZ[:, :, 4 + OFF[i]:4 + OFF[i] + w]
                        T(prun, prun, tmp[:, :, 0:w], SUB)

                # ---- downcast + DMA out --------------------------------
                nc.scalar.copy(OUTB[:, :, 0:8], DX)
                nc.scalar.copy(OUTB[:, :, 8:44], XPZ[:, :, 4:40])
                nc.sync.dma_start(out=ov[:, FS, :], in_=OUTB)

    nc.compile()
    return nc


_cache = {}


def _get_runner():
    """Build the Bass program and a persistent jitted shard_map executor
    (the same bass_exec-primitive path run_bass_kernel_spmd takes under
    axon, kept cached across calls)."""
    if "fn" in _cache:
        return _cache

    import jax
    from jax.sharding import Mesh, PartitionSpec, NamedSharding
    from jax.experimental.shard_map import shard_map
    from concourse.bass2jax import (_bass_exec_p, partition_id_tensor,
                                    install_neuronx_cc_hook)

    install_neuronx_cc_hook()
    nc = _build_program(BC)

    partition_name = (nc.partition_id_tensor.name
                      if nc.partition_id_tensor else None)
    in_names, out_names, out_avals = [], [], []
    for alloc in nc.m.functions[0].allocations:
        if not isinstance(alloc, mybir.MemoryLocationSet):
            continue
        name = alloc.memorylocations[0].name
        if alloc.kind == "ExternalInput":
            if name != partition_name:
                in_names.append(name)
        elif alloc.kind == "ExternalOutput":
            out_avals.append(jax.core.ShapedArray(
                tuple(alloc.tensor_shape), mybir.dt.np(alloc.dtype)))
            out_names.append(name)
    bind_names = list(in_names)
    if partition_name is not None:
        bind_names.append(partition_name)

    def _body(*args):
        operands = list(args)
        if partition_name is not None:
            operands.append(partition_id_tensor())
        outs = _bass_exec_p.bind(
            *operands, out_avals=tuple(out_avals), in_names=tuple(bind_names),
            out_names=tuple(out_names), lowering_input_output_aliases=(),
            sim_require_finite=True, sim_require_nnan=True, nc=nc)
        return tuple(outs)

    devices = jax.devices()[:NCORES]
    mesh = Mesh(np.asarray(devices), ("core",))
    fn = jax.jit(shard_map(
        _body, mesh=mesh, in_specs=(PartitionSpec("core"),) * len(in_names),
        out_specs=(PartitionSpec("core"),) * len(out_names), check_rep=False))

    sharding = NamedSharding(mesh, PartitionSpec("core"))
    idcat = jax.device_put(
        np.tile(np.eye(128, dtype=np.float32), (NCORES, 1)), sharding)
    idcat.block_until_ready()

    _cache.update(fn=fn, in_names=in_names, mesh=mesh, sharding=sharding,
                  jax=jax, idcat=idcat,
                  pool=ThreadPoolExecutor(max_workers=8),
                  cpool=ThreadPoolExecutor(max_workers=4), w2={})
    return _cache


def _fetch(pool, arr):
    """Parallel per-shard d2h: the tunnel does ~28 MB/s on one stream but
    ~50 MB/s on eight."""
    shards = sorted(arr.addressable_shards,
                    key=lambda s: s.index[0].start or 0)
    parts = list(pool.map(lambda s: np.asarray(s.data), shards))
    return np.concatenate(parts, axis=0)


def kernel(x: np.ndarray, z: np.ndarray, P: np.ndarray,
           H: np.ndarray, R: np.ndarray) -> np.ndarray:
    st = _get_runner()
    jax = st["jax"]

    H = np.asarray(H, np.float32)
    R = np.asarray(R, np.float32)
    xr = np.ascontiguousarray(x, dtype=np.float32).reshape(B, 8)
    zr = np.ascontiguousarray(z, dtype=np.float32).reshape(B, 4)
    Pr = np.ascontiguousarray(P, dtype=np.float32).reshape(B, 64)

    key = (H.tobytes(), R.tobytes())
    if key not in st["w2"]:
        st["w2"].clear()
        st["w2"][key] = jax.device_put(
            np.tile(_build_w2(H, R), (NCORES, 1)), st["sharding"])
    w2cat = st["w2"][key]
    HT = H.T.copy()

    # pack + dispatch chunk-by-chunk so chunk c+1's host packing overlaps
    # chunk c's (async) upload; fetches then pipeline through the pool.
    outs = []
    rows = B // CHUNKS
    for c in range(CHUNKS):
        sl = slice(c * rows, (c + 1) * rows)
        y16 = (zr[sl] - xr[sl] @ HT).astype(np.float16)
        pu16 = Pr[sl, :][:, IU].astype(np.float16)
        args = []
        for nm in st["in_names"]:
            if nm == "yd":
                args.append(y16)
            elif nm == "pud":
                args.append(pu16)
            elif nm == "w2d":
                args.append(w2cat)
            elif nm == "idd":
                args.append(st["idcat"])
            else:
                raise KeyError(nm)
        outs.append(st["fn"](*args)[0])

    res72 = np.empty((B, 72), dtype=np.float32)
    PIDX = 8 + FULLIDX
    futs = [st["cpool"].submit(_fetch, st["pool"], o) for o in outs]
    for c in range(CHUNKS):
        sl = slice(c * rows, (c + 1) * rows)
        out44 = futs[c].result()                      # [rows, 44] fp16
        res72[sl, 0:8] = xr[sl] + out44[:, 0:8]
        res72[sl, 8:72] = out44[:, PIDX]
    return res72.reshape(B, 9, 8)
